# revision 8
# baseline (speedup 1.0000x reference)
"""Trainium2 Bass kernel for the LELoss problem (raw Bass, 8-core SPMD).

loss = mean_b ||x_b - dec_b||^2
     + 1.1 * mean_b ||enc_b - (lat @ rsrA.T)_b||^2
     + 0.1 * mean((rsrA.T @ rsrA - I)^2)

(The knn/cdist/topk in the original module is dead code - its result is never
used - so the returned loss reduces to the three terms above.)

Per-core algebra (batch shard of R=1024 rows):
  sum||enc - lat@A.T||^2 = sum(enc^2) - 2*sum(M .* A) + sum(L .* G0)
      with M = enc.T @ lat [E,I], L = lat.T @ lat [I,I], G0 = A.T @ A [I,I]
  sum((G0 - I)^2) = sum(G0^2) - 2*sum(A^2) + I_dim
All partial sums land in columns of a [128,16] SBUF accumulator S which is
DMA'd out per core; the host collapses partitions/cores and applies weights.

DMA strategy: the two HWDGE queues (SP and ACT engines) each stream ~4.3MB so
the ~415 GB/s/core HBM path is the only limiter. enc and lat use a
contiguous-rows layout ("(p n) d -> p n d": partition p holds rows
8p..8p+7) giving 4KB/640B descriptors - M and L are sums over all rows, so
any partition<->row assignment works as long as enc and lat share it. x tile
6 is row-split across both queues to balance them exactly; the last tile's
subtract AND square both run on DVE so the tail has no cross-engine hop.
"""

import contextlib

import numpy as np

try:
    import concourse.bass as bass
except ImportError:  # pragma: no cover - grading env fallback
    import sys

    sys.path.insert(0, "/opt/trn_rl_repo")
    import concourse.bass as bass

from concourse import mybir
from concourse.bass_utils import run_bass_kernel_spmd

N_CORES = 8
B, D, E, I = 8192, 1024, 128, 20
R = B // N_CORES  # rows per core = 1024
P = 128  # SBUF partitions
RT = R // P  # row tiles per core = 8
S_COLS = 16
F32 = mybir.dt.float32

TRACE = False
LAST_RESULT = None

_NC = None


def _build_nc():
    nc = bass.Bass()
    x = nc.dram_tensor("x", [R, D], F32, kind="ExternalInput")
    dec = nc.dram_tensor("dec", [R, D], F32, kind="ExternalInput")
    enc = nc.dram_tensor("enc", [R, E], F32, kind="ExternalInput")
    lat = nc.dram_tensor("lat", [R, I], F32, kind="ExternalInput")
    rsra = nc.dram_tensor("rsra", [E, I], F32, kind="ExternalInput")
    out = nc.dram_tensor("out", [P, S_COLS], F32, kind="ExternalOutput")

    Square = mybir.ActivationFunctionType.Square
    mult = mybir.AluOpType.mult
    bypass = mybir.AluOpType.bypass

    # contiguous-rows layouts (partition p holds rows 8p..8p+7)
    enc_r = enc[:, :].rearrange("(p n) e -> p n e", p=P)  # 4KB chunks
    lat_r = lat[:, :].rearrange("(p n) i -> p n i", p=P)  # 640B chunks

    ctx = contextlib.ExitStack()
    with ctx:
        xb = [
            ctx.enter_context(nc.sbuf_tensor(f"xb{t}", [P, D], F32)) for t in range(RT)
        ]
        db = [
            ctx.enter_context(nc.sbuf_tensor(f"db{t}", [P, D], F32)) for t in range(RT)
        ]
        enc_sb = ctx.enter_context(nc.sbuf_tensor([P, RT * E], F32))
        lat_sb = ctx.enter_context(nc.sbuf_tensor([P, RT * I], F32))
        rsra_sb = ctx.enter_context(nc.sbuf_tensor([E, I], F32))
        S = ctx.enter_context(nc.sbuf_tensor([P, S_COLS], F32))
        G_sb = ctx.enter_context(nc.sbuf_tensor([I, I], F32))
        scr_m = ctx.enter_context(nc.sbuf_tensor([E, I], F32))
        scr_i = ctx.enter_context(nc.sbuf_tensor([I, I], F32))
        scr_a = ctx.enter_context(nc.sbuf_tensor([E, I], F32))
        scr_e = ctx.enter_context(nc.sbuf_tensor([P, RT * E], F32))

        psum_M = ctx.enter_context(nc.psum_tensor([E, I], F32))
        psum_L = ctx.enter_context(nc.psum_tensor([I, I], F32))
        psum_G = ctx.enter_context(nc.psum_tensor([I, I], F32))

        s_x = [ctx.enter_context(nc.semaphore(f"s_x{t}")) for t in range(RT)]
        s_small = ctx.enter_context(nc.semaphore("s_small"))
        s_init = ctx.enter_context(nc.semaphore("s_init"))
        s_sub = ctx.enter_context(nc.semaphore("s_sub"))
        s_sq = ctx.enter_context(nc.semaphore("s_sq"))
        s_pe = ctx.enter_context(nc.semaphore("s_pe"))
        s_vfin = ctx.enter_context(nc.semaphore("s_vfin"))
        s_out = ctx.enter_context(nc.semaphore("s_out"))

        block = ctx.enter_context(nc.Block())

        RH = P // 2  # row half of a tile

        @block.sync
        def _(sync):
            # SP HWDGE queue (~4.25MB): enc, x0..x5, x6 first row-half, x7
            sync.dma_start(
                out=enc_sb[:, :].rearrange("p (n e) -> p n e", n=RT), in_=enc_r
            ).then_inc(s_small, 16)
            for t in range(RT - 2):
                sync.dma_start(
                    out=xb[t][:, :], in_=x[t * P : (t + 1) * P, :]
                ).then_inc(s_x[t], 16)
            sync.dma_start(
                out=xb[6][0:RH, :], in_=x[6 * P : 6 * P + RH, :]
            ).then_inc(s_x[6], 16)
            sync.dma_start(
                out=xb[7][:, :], in_=x[7 * P : 8 * P, :]
            ).then_inc(s_x[7], 16)
            # ship the accumulator once every column is final
            sync.wait_ge(s_sq, 9)
            sync.wait_ge(s_vfin, 2)
            sync.dma_start(out=out[:, :], in_=S[:, :]).then_inc(s_out, 16)
            sync.wait_ge(s_out, 16)

        @block.scalar
        def _(scalar):
            # ACT HWDGE queue (~4.34MB): rsra, lat, dec0..dec6, x6 second
            # row-half, dec7
            scalar.dma_start(out=rsra_sb[:, :], in_=rsra[:, :]).then_inc(s_small, 16)
            scalar.dma_start(
                out=lat_sb[:, :].rearrange("p (n i) -> p n i", n=RT), in_=lat_r
            ).then_inc(s_small, 16)
            for t in range(RT - 1):
                scalar.dma_start(
                    out=db[t][:, :], in_=dec[t * P : (t + 1) * P, :]
                ).then_inc(s_x[t], 16)
            scalar.dma_start(
                out=xb[6][RH:P, :], in_=x[6 * P + RH : 7 * P, :]
            ).then_inc(s_x[6], 16)
            scalar.dma_start(
                out=db[7][:, :], in_=dec[7 * P : 8 * P, :]
            ).then_inc(s_x[7], 16)
            # squares of the streamed differences (tiles 0..6)
            scalar.wait_ge(s_init, 1)
            for t in range(RT - 1):
                scalar.wait_ge(s_sub, t + 1)
                nc.scalar.activation(
                    out=db[t][:, :], in_=xb[t][:, :], func=Square,
                    accum_out=S[:, t : t + 1],
                ).then_inc(s_sq, 1)
                if t == 1:
                    # fill the idle gap with the small squares
                    scalar.wait_ge(s_small, 48)
                    nc.scalar.activation(
                        out=scr_e[:, :], in_=enc_sb[:, :], func=Square,
                        accum_out=S[:, 8:9],
                    ).then_inc(s_sq, 1)
                    nc.scalar.activation(
                        out=scr_a[:, :], in_=rsra_sb[:, :], func=Square,
                        accum_out=S[:E, 12:13],
                    ).then_inc(s_sq, 1)

        @block.vector
        def _(vector):
            nc.vector.memset(S[:, :], 0.0).then_inc(s_init, 1)
            # the big stream: d = x - dec, in place
            for t in range(5):
                vector.wait_ge(s_x[t], 32)
                nc.vector.tensor_sub(xb[t][:, :], xb[t][:, :], db[t][:, :]).then_inc(
                    s_sub, 1
                )
            # tiny fused reductions over the PCA/proj matmul results, slotted
            # into the stream's idle gap
            vector.wait_ge(s_pe, 1)
            nc.vector.tensor_copy(G_sb[:, :], psum_G[:, :])
            nc.vector.scalar_tensor_tensor(
                out=scr_m[:, :], in0=psum_M[:, :], scalar=1.0, in1=rsra_sb[:, :],
                op0=bypass, op1=mult, accum_out=S[:E, 9:10],
            )
            nc.vector.scalar_tensor_tensor(
                out=scr_i[:, :], in0=psum_L[:, :], scalar=1.0, in1=G_sb[:, :],
                op0=bypass, op1=mult, accum_out=S[:I, 10:11],
            )
            nc.vector.scalar_tensor_tensor(
                out=scr_i[:, :], in0=G_sb[:, :], scalar=1.0, in1=G_sb[:, :],
                op0=bypass, op1=mult, accum_out=S[:I, 11:12],
            ).then_inc(s_vfin, 1)
            vector.wait_ge(s_x[5], 32)
            nc.vector.tensor_sub(xb[5][:, :], xb[5][:, :], db[5][:, :]).then_inc(
                s_sub, 1
            )
            vector.wait_ge(s_x[6], 48)  # two x row-halves + dec6
            nc.vector.tensor_sub(xb[6][:, :], xb[6][:, :], db[6][:, :]).then_inc(
                s_sub, 1
            )
            # last tile entirely on DVE: subtract then fused square-accumulate
            vector.wait_ge(s_x[7], 32)
            nc.vector.tensor_sub(xb[7][:, :], xb[7][:, :], db[7][:, :])
            nc.vector.scalar_tensor_tensor(
                out=db[7][:, :], in0=xb[7][:, :], scalar=1.0, in1=xb[7][:, :],
                op0=bypass, op1=mult, accum_out=S[:, 13:14],
            ).then_inc(s_vfin, 1)

        @block.tensor
        def _(tensor):
            tensor.wait_ge(s_small, 48)
            for t in range(RT):
                nc.tensor.matmul(
                    psum_M[:, :],
                    lhsT=enc_sb[:, t * E : (t + 1) * E],
                    rhs=lat_sb[:, t * I : (t + 1) * I],
                    start=(t == 0),
                    stop=(t == RT - 1),
                )
            for t in range(RT):
                nc.tensor.matmul(
                    psum_L[:, :],
                    lhsT=lat_sb[:, t * I : (t + 1) * I],
                    rhs=lat_sb[:, t * I : (t + 1) * I],
                    start=(t == 0),
                    stop=(t == RT - 1),
                )
            nc.tensor.matmul(
                psum_G[:, :], lhsT=rsra_sb[:, :], rhs=rsra_sb[:, :],
                start=True, stop=True,
            ).then_inc(s_pe, 1)

    return nc


def kernel(x, encoded, latent, decoded, rsrA):
    global _NC, LAST_RESULT
    if _NC is None:
        _NC = _build_nc()

    x = np.ascontiguousarray(x, dtype=np.float32)
    decoded = np.ascontiguousarray(decoded, dtype=np.float32)
    encoded = np.ascontiguousarray(encoded, dtype=np.float32)
    latent = np.ascontiguousarray(latent, dtype=np.float32)
    rsrA = np.ascontiguousarray(rsrA, dtype=np.float32)

    in_maps = []
    for c in range(N_CORES):
        sl = slice(c * R, (c + 1) * R)
        in_maps.append(
            {
                "x": x[sl],
                "dec": decoded[sl],
                "enc": encoded[sl],
                "lat": latent[sl],
                "rsra": rsrA,
            }
        )

    res = run_bass_kernel_spmd(_NC, in_maps, core_ids=list(range(N_CORES)), trace=TRACE)
    LAST_RESULT = res

    o = np.stack([r["out"] for r in res.results]).astype(np.float64)  # [8,128,16]
    cols = o.sum(axis=(0, 1))  # [16]
    s_recon = cols[0:8].sum() + cols[13]
    s_enc2 = cols[8]
    s_cross = cols[9]
    s_zsq = cols[10]
    g2 = o[0, :, 11].sum()
    ra2 = o[0, :, 12].sum()

    pca_sq = s_enc2 - 2.0 * s_cross + s_zsq
    proj_sq = g2 - 2.0 * ra2 + float(I)
    loss = s_recon / B + 1.1 * pca_sq / B + 0.1 * proj_sq / (I * I)
    return np.asarray(loss, dtype=np.float32)


# revision 9
# speedup vs baseline: 1.0868x; 1.0868x over previous
"""Trainium2 Bass kernel for the LELoss problem (raw Bass, 8-core SPMD).

loss = mean_b ||x_b - dec_b||^2
     + 1.1 * mean_b ||enc_b - (lat @ rsrA.T)_b||^2
     + 0.1 * mean((rsrA.T @ rsrA - I)^2)

(The knn/cdist/topk in the original module is dead code - its result is never
used - so the returned loss reduces to the three terms above.)

Per-core algebra (batch shard of R=1024 rows):
  sum||enc - lat@A.T||^2 = sum(enc^2) - 2*sum(M .* A) + sum(L .* G0)
      with M = enc.T @ lat [E,I], L = lat.T @ lat [I,I], G0 = A.T @ A [I,I]
  sum((G0 - I)^2) = sum(G0^2) - 2*sum(A^2) + I_dim
All partial sums land in columns of a [128,16] SBUF accumulator S which is
DMA'd out per core; the host collapses partitions/cores and applies weights.

DMA strategy: the two HWDGE queues (SP and ACT engines) each stream ~4.3MB
of 4KB-chunk transfers so the ~415 GB/s/core HBM path is the only limiter.
enc/lat/rsrA are pre-packed on the host (pure reshape/concat, no arithmetic)
into one [128, 1204] array whose rows are the exact SBUF partition images
(partition p holds enc rows 8p..8p+7, lat rows 8p..8p+7, rsrA row p), so the
small tensors ride a single full-rate DMA instead of many tiny-descriptor
ones. M and L are sums over all rows, so this partition<->row assignment is
valid as long as enc and lat share it (they do). x tile 6 is row-split
across both queues to balance them; the last tile's subtract AND
square-accumulate both run on DVE so the tail has no cross-engine hop.
"""

import contextlib

import numpy as np

try:
    import concourse.bass as bass
except ImportError:  # pragma: no cover - grading env fallback
    import sys

    sys.path.insert(0, "/opt/trn_rl_repo")
    import concourse.bass as bass

from concourse import mybir
from concourse.bass_utils import run_bass_kernel_spmd

N_CORES = 8
B, D, E, I = 8192, 1024, 128, 20
R = B // N_CORES  # rows per core = 1024
P = 128  # SBUF partitions
RT = R // P  # row tiles per core = 8
S_COLS = 16
F32 = mybir.dt.float32

ENC_W = RT * E  # 1024 cols of packed enc
LAT_W = RT * I  # 160 cols of packed lat
PACK_W = ENC_W + LAT_W + I  # 1204

TRACE = False
LAST_RESULT = None

_NC = None


def _build_nc():
    nc = bass.Bass()
    x = nc.dram_tensor("x", [R, D], F32, kind="ExternalInput")
    dec = nc.dram_tensor("dec", [R, D], F32, kind="ExternalInput")
    pack = nc.dram_tensor("pack", [P, PACK_W], F32, kind="ExternalInput")
    out = nc.dram_tensor("out", [P, S_COLS], F32, kind="ExternalOutput")

    Square = mybir.ActivationFunctionType.Square
    mult = mybir.AluOpType.mult
    bypass = mybir.AluOpType.bypass

    ctx = contextlib.ExitStack()
    with ctx:
        xb = [
            ctx.enter_context(nc.sbuf_tensor(f"xb{t}", [P, D], F32)) for t in range(RT)
        ]
        db = [
            ctx.enter_context(nc.sbuf_tensor(f"db{t}", [P, D], F32)) for t in range(RT)
        ]
        small_sb = ctx.enter_context(nc.sbuf_tensor([P, PACK_W], F32))
        S = ctx.enter_context(nc.sbuf_tensor([P, S_COLS], F32))
        G_sb = ctx.enter_context(nc.sbuf_tensor([I, I], F32))
        scr_m = ctx.enter_context(nc.sbuf_tensor([E, I], F32))
        scr_i = ctx.enter_context(nc.sbuf_tensor([I, I], F32))
        scr_a = ctx.enter_context(nc.sbuf_tensor([E, I], F32))
        scr_e = ctx.enter_context(nc.sbuf_tensor([P, ENC_W], F32))

        psum_M = ctx.enter_context(nc.psum_tensor([E, I], F32))
        psum_L = ctx.enter_context(nc.psum_tensor([I, I], F32))
        psum_G = ctx.enter_context(nc.psum_tensor([I, I], F32))

        s_x = [ctx.enter_context(nc.semaphore(f"s_x{t}")) for t in range(RT)]
        s_small = ctx.enter_context(nc.semaphore("s_small"))
        s_init = ctx.enter_context(nc.semaphore("s_init"))
        s_sub = ctx.enter_context(nc.semaphore("s_sub"))
        s_sq = ctx.enter_context(nc.semaphore("s_sq"))
        s_pe = ctx.enter_context(nc.semaphore("s_pe"))
        s_vfin = ctx.enter_context(nc.semaphore("s_vfin"))
        s_out = ctx.enter_context(nc.semaphore("s_out"))

        block = ctx.enter_context(nc.Block())

        RH = P // 2  # row half of a tile

        def enc_t(t):
            return small_sb[:, t * E : (t + 1) * E]

        def lat_t(t):
            return small_sb[:, ENC_W + t * I : ENC_W + (t + 1) * I]

        rsra_sb = small_sb[:, ENC_W + LAT_W : PACK_W]

        @block.sync
        def _(sync):
            # SP HWDGE queue (~4.34MB): pack, x0..x5, x6 first row-half, x7
            sync.dma_start(out=small_sb[:, :], in_=pack[:, :]).then_inc(s_small, 16)
            for t in range(RT - 2):
                sync.dma_start(
                    out=xb[t][:, :], in_=x[t * P : (t + 1) * P, :]
                ).then_inc(s_x[t], 16)
            sync.dma_start(
                out=xb[6][0:RH, :], in_=x[6 * P : 6 * P + RH, :]
            ).then_inc(s_x[6], 16)
            sync.dma_start(
                out=xb[7][:, :], in_=x[7 * P : 8 * P, :]
            ).then_inc(s_x[7], 16)
            # ship the accumulator once every column is final
            sync.wait_ge(s_sq, 9)
            sync.wait_ge(s_vfin, 2)
            sync.dma_start(out=out[:, :], in_=S[:, :]).then_inc(s_out, 16)
            sync.wait_ge(s_out, 16)

        @block.scalar
        def _(scalar):
            # ACT HWDGE queue (~4.25MB): dec0..dec6, x6 second row-half, dec7
            for t in range(RT - 1):
                scalar.dma_start(
                    out=db[t][:, :], in_=dec[t * P : (t + 1) * P, :]
                ).then_inc(s_x[t], 16)
            scalar.dma_start(
                out=xb[6][RH:P, :], in_=x[6 * P + RH : 7 * P, :]
            ).then_inc(s_x[6], 16)
            scalar.dma_start(
                out=db[7][:, :], in_=dec[7 * P : 8 * P, :]
            ).then_inc(s_x[7], 16)
            # squares of the streamed differences (tiles 0..6)
            scalar.wait_ge(s_init, 1)
            for t in range(RT - 1):
                scalar.wait_ge(s_sub, t + 1)
                nc.scalar.activation(
                    out=db[t][:, :], in_=xb[t][:, :], func=Square,
                    accum_out=S[:, t : t + 1],
                ).then_inc(s_sq, 1)
                if t == 1:
                    # fill the idle gap with the small squares
                    scalar.wait_ge(s_small, 16)
                    nc.scalar.activation(
                        out=scr_e[:, :], in_=small_sb[:, 0:ENC_W], func=Square,
                        accum_out=S[:, 8:9],
                    ).then_inc(s_sq, 1)
                    nc.scalar.activation(
                        out=scr_a[:, :], in_=rsra_sb, func=Square,
                        accum_out=S[:E, 12:13],
                    ).then_inc(s_sq, 1)

        @block.vector
        def _(vector):
            nc.vector.memset(S[:, :], 0.0).then_inc(s_init, 1)
            # the big stream: d = x - dec, in place
            for t in range(5):
                vector.wait_ge(s_x[t], 32)
                nc.vector.tensor_sub(xb[t][:, :], xb[t][:, :], db[t][:, :]).then_inc(
                    s_sub, 1
                )
            # tiny fused reductions over the PCA/proj matmul results, slotted
            # into the stream's idle gap
            vector.wait_ge(s_pe, 1)
            nc.vector.tensor_copy(G_sb[:, :], psum_G[:, :])
            nc.vector.scalar_tensor_tensor(
                out=scr_m[:, :], in0=psum_M[:, :], scalar=1.0, in1=rsra_sb,
                op0=bypass, op1=mult, accum_out=S[:E, 9:10],
            )
            nc.vector.scalar_tensor_tensor(
                out=scr_i[:, :], in0=psum_L[:, :], scalar=1.0, in1=G_sb[:, :],
                op0=bypass, op1=mult, accum_out=S[:I, 10:11],
            )
            nc.vector.scalar_tensor_tensor(
                out=scr_i[:, :], in0=G_sb[:, :], scalar=1.0, in1=G_sb[:, :],
                op0=bypass, op1=mult, accum_out=S[:I, 11:12],
            ).then_inc(s_vfin, 1)
            vector.wait_ge(s_x[5], 32)
            nc.vector.tensor_sub(xb[5][:, :], xb[5][:, :], db[5][:, :]).then_inc(
                s_sub, 1
            )
            vector.wait_ge(s_x[6], 48)  # two x row-halves + dec6
            nc.vector.tensor_sub(xb[6][:, :], xb[6][:, :], db[6][:, :]).then_inc(
                s_sub, 1
            )
            # last tile entirely on DVE: subtract then fused square-accumulate
            vector.wait_ge(s_x[7], 32)
            nc.vector.tensor_sub(xb[7][:, :], xb[7][:, :], db[7][:, :])
            nc.vector.scalar_tensor_tensor(
                out=db[7][:, :], in0=xb[7][:, :], scalar=1.0, in1=xb[7][:, :],
                op0=bypass, op1=mult, accum_out=S[:, 13:14],
            ).then_inc(s_vfin, 1)

        @block.tensor
        def _(tensor):
            tensor.wait_ge(s_small, 16)
            for t in range(RT):
                nc.tensor.matmul(
                    psum_M[:, :], lhsT=enc_t(t), rhs=lat_t(t),
                    start=(t == 0), stop=(t == RT - 1),
                )
            for t in range(RT):
                nc.tensor.matmul(
                    psum_L[:, :], lhsT=lat_t(t), rhs=lat_t(t),
                    start=(t == 0), stop=(t == RT - 1),
                )
            nc.tensor.matmul(
                psum_G[:, :], lhsT=rsra_sb, rhs=rsra_sb, start=True, stop=True
            ).then_inc(s_pe, 1)

    return nc


def kernel(x, encoded, latent, decoded, rsrA):
    global _NC, LAST_RESULT
    if _NC is None:
        _NC = _build_nc()

    x = np.ascontiguousarray(x, dtype=np.float32)
    decoded = np.ascontiguousarray(decoded, dtype=np.float32)
    encoded = np.ascontiguousarray(encoded, dtype=np.float32)
    latent = np.ascontiguousarray(latent, dtype=np.float32)
    rsrA = np.ascontiguousarray(rsrA, dtype=np.float32)

    in_maps = []
    for c in range(N_CORES):
        sl = slice(c * R, (c + 1) * R)
        # host-side packing: pure layout transform (reshape/concat), the rows
        # are the per-partition SBUF images for the small tensors
        pk = np.concatenate(
            [
                encoded[sl].reshape(P, ENC_W),
                latent[sl].reshape(P, LAT_W),
                rsrA,
            ],
            axis=1,
        )
        in_maps.append({"x": x[sl], "dec": decoded[sl], "pack": pk})

    res = run_bass_kernel_spmd(_NC, in_maps, core_ids=list(range(N_CORES)), trace=TRACE)
    LAST_RESULT = res

    o = np.stack([r["out"] for r in res.results]).astype(np.float64)  # [8,128,16]
    cols = o.sum(axis=(0, 1))  # [16]
    s_recon = cols[0:8].sum() + cols[13]
    s_enc2 = cols[8]
    s_cross = cols[9]
    s_zsq = cols[10]
    g2 = o[0, :, 11].sum()
    ra2 = o[0, :, 12].sum()

    pca_sq = s_enc2 - 2.0 * s_cross + s_zsq
    proj_sq = g2 - 2.0 * ra2 + float(I)
    loss = s_recon / B + 1.1 * pca_sq / B + 0.1 * proj_sq / (I * I)
    return np.asarray(loss, dtype=np.float32)


# revision 10
# speedup vs baseline: 1.1258x; 1.0359x over previous
"""Trainium2 Bass kernel for the LELoss problem (raw Bass, 8-core SPMD).

loss = mean_b ||x_b - dec_b||^2
     + 1.1 * mean_b ||enc_b - (lat @ rsrA.T)_b||^2
     + 0.1 * mean((rsrA.T @ rsrA - I)^2)

(The knn/cdist/topk in the original module is dead code - its result is never
used - so the returned loss reduces to the three terms above.)

Per-core algebra (batch shard of R=1024 rows):
  sum||enc - lat@A.T||^2 = sum(enc^2) - 2*sum(M .* A) + sum(L .* G0)
      with M = enc.T @ lat [E,I], L = lat.T @ lat [I,I], G0 = A.T @ A [I,I]
  sum((G0 - I)^2) = sum(G0^2) - 2*sum(A^2) + I_dim
All partial sums land in columns of a [128,16] SBUF accumulator S which is
DMA'd out per core; the host collapses partitions/cores and applies weights.

DMA strategy: the two HWDGE queues (SP and ACT engines) each stream ~4.3MB
of >=2KB-chunk transfers so the ~415 GB/s/core HBM path is the only limiter.
enc/lat/rsrA are pre-packed on the host (pure reshape/concat, no arithmetic)
into one [128, 1204] array whose rows are the exact SBUF partition images
(partition p holds enc rows 8p..8p+7, lat rows 8p..8p+7, rsrA row p); that
pack rides mid-queue since the matmuls have slack. x tile 6 is row-split
across both queues for balance; tile 7 is column-split so its two halves
pipeline through the subtract/square tail.
"""

import contextlib

import numpy as np

try:
    import concourse.bass as bass
except ImportError:  # pragma: no cover - grading env fallback
    import sys

    sys.path.insert(0, "/opt/trn_rl_repo")
    import concourse.bass as bass

from concourse import mybir
from concourse.bass_utils import run_bass_kernel_spmd

N_CORES = 8
B, D, E, I = 8192, 1024, 128, 20
R = B // N_CORES  # rows per core = 1024
P = 128  # SBUF partitions
RT = R // P  # row tiles per core = 8
S_COLS = 16
F32 = mybir.dt.float32

ENC_W = RT * E  # 1024 cols of packed enc
LAT_W = RT * I  # 160 cols of packed lat
PACK_W = ENC_W + LAT_W + I  # 1204

TRACE = False
LAST_RESULT = None

_NC = None


def _build_nc():
    nc = bass.Bass()
    x = nc.dram_tensor("x", [R, D], F32, kind="ExternalInput")
    dec = nc.dram_tensor("dec", [R, D], F32, kind="ExternalInput")
    pack = nc.dram_tensor("pack", [P, PACK_W], F32, kind="ExternalInput")
    out = nc.dram_tensor("out", [P, S_COLS], F32, kind="ExternalOutput")

    Square = mybir.ActivationFunctionType.Square
    mult = mybir.AluOpType.mult
    bypass = mybir.AluOpType.bypass

    ctx = contextlib.ExitStack()
    with ctx:
        xb = [
            ctx.enter_context(nc.sbuf_tensor(f"xb{t}", [P, D], F32)) for t in range(RT)
        ]
        db = [
            ctx.enter_context(nc.sbuf_tensor(f"db{t}", [P, D], F32)) for t in range(RT)
        ]
        small_sb = ctx.enter_context(nc.sbuf_tensor([P, PACK_W], F32))
        S = ctx.enter_context(nc.sbuf_tensor([P, S_COLS], F32))
        G_sb = ctx.enter_context(nc.sbuf_tensor([I, I], F32))
        scr_m = ctx.enter_context(nc.sbuf_tensor([E, I], F32))
        scr_i = ctx.enter_context(nc.sbuf_tensor([I, I], F32))
        scr_a = ctx.enter_context(nc.sbuf_tensor([E, I], F32))
        scr_e = ctx.enter_context(nc.sbuf_tensor([P, ENC_W], F32))

        psum_M = ctx.enter_context(nc.psum_tensor([E, I], F32))
        psum_L = ctx.enter_context(nc.psum_tensor([I, I], F32))
        psum_G = ctx.enter_context(nc.psum_tensor([I, I], F32))

        # pair sems: 0..6 row tiles, 7 = tile7 cols 0:512, 8 = cols 512:1024
        s_x = [ctx.enter_context(nc.semaphore(f"s_x{t}")) for t in range(RT + 1)]
        s_small = ctx.enter_context(nc.semaphore("s_small"))
        s_init = ctx.enter_context(nc.semaphore("s_init"))
        s_sub = ctx.enter_context(nc.semaphore("s_sub"))
        s_sq = ctx.enter_context(nc.semaphore("s_sq"))
        s_pe = ctx.enter_context(nc.semaphore("s_pe"))
        s_vfin = ctx.enter_context(nc.semaphore("s_vfin"))
        s_out = ctx.enter_context(nc.semaphore("s_out"))

        block = ctx.enter_context(nc.Block())

        RH = P // 2  # row half
        DH = D // 2  # column half

        def enc_t(t):
            return small_sb[:, t * E : (t + 1) * E]

        def lat_t(t):
            return small_sb[:, ENC_W + t * I : ENC_W + (t + 1) * I]

        rsra_sb = small_sb[:, ENC_W + LAT_W : PACK_W]

        @block.sync
        def _(sync):
            # SP HWDGE queue (~4.34MB): x0, x1, pack, x2..x5, x6 first
            # row-half, x7 column halves
            for t in range(2):
                sync.dma_start(
                    out=xb[t][:, :], in_=x[t * P : (t + 1) * P, :]
                ).then_inc(s_x[t], 16)
            sync.dma_start(out=small_sb[:, :], in_=pack[:, :]).then_inc(s_small, 16)
            for t in range(2, RT - 2):
                sync.dma_start(
                    out=xb[t][:, :], in_=x[t * P : (t + 1) * P, :]
                ).then_inc(s_x[t], 16)
            sync.dma_start(
                out=xb[6][0:RH, :], in_=x[6 * P : 6 * P + RH, :]
            ).then_inc(s_x[6], 16)
            sync.dma_start(
                out=xb[7][:, 0:DH], in_=x[7 * P : 8 * P, 0:DH]
            ).then_inc(s_x[7], 16)
            sync.dma_start(
                out=xb[7][:, DH:D], in_=x[7 * P : 8 * P, DH:D]
            ).then_inc(s_x[8], 16)
            # ship the accumulator once every column is final
            sync.wait_ge(s_sq, 10)
            sync.wait_ge(s_vfin, 2)
            sync.dma_start(out=out[:, :], in_=S[:, :]).then_inc(s_out, 16)
            sync.wait_ge(s_out, 16)

        @block.scalar
        def _(scalar):
            # ACT HWDGE queue (~4.25MB): dec0..dec6, x6 second row-half,
            # dec7 column halves
            for t in range(RT - 1):
                scalar.dma_start(
                    out=db[t][:, :], in_=dec[t * P : (t + 1) * P, :]
                ).then_inc(s_x[t], 16)
            scalar.dma_start(
                out=xb[6][RH:P, :], in_=x[6 * P + RH : 7 * P, :]
            ).then_inc(s_x[6], 16)
            scalar.dma_start(
                out=db[7][:, 0:DH], in_=dec[7 * P : 8 * P, 0:DH]
            ).then_inc(s_x[7], 16)
            scalar.dma_start(
                out=db[7][:, DH:D], in_=dec[7 * P : 8 * P, DH:D]
            ).then_inc(s_x[8], 16)
            # squares of the streamed differences (tiles 0..6 and 7 cols 0:512)
            scalar.wait_ge(s_init, 1)
            for t in range(RT - 1):
                scalar.wait_ge(s_sub, t + 1)
                nc.scalar.activation(
                    out=db[t][:, :], in_=xb[t][:, :], func=Square,
                    accum_out=S[:, t : t + 1],
                ).then_inc(s_sq, 1)
                if t == 1:
                    scalar.wait_ge(s_small, 16)
                    nc.scalar.activation(
                        out=scr_e[:, :], in_=small_sb[:, 0:ENC_W], func=Square,
                        accum_out=S[:, 8:9],
                    ).then_inc(s_sq, 1)
                    nc.scalar.activation(
                        out=scr_a[:, :], in_=rsra_sb, func=Square,
                        accum_out=S[:E, 12:13],
                    ).then_inc(s_sq, 1)
            scalar.wait_ge(s_sub, 8)
            nc.scalar.activation(
                out=db[7][:, 0:DH], in_=xb[7][:, 0:DH], func=Square,
                accum_out=S[:, 7:8],
            ).then_inc(s_sq, 1)

        @block.vector
        def _(vector):
            nc.vector.memset(S[:, :], 0.0).then_inc(s_init, 1)
            # the big stream: d = x - dec, in place
            for t in range(RT - 1):
                vector.wait_ge(s_x[t], 32 if t != 6 else 48)
                nc.vector.tensor_sub(xb[t][:, :], xb[t][:, :], db[t][:, :]).then_inc(
                    s_sub, 1
                )
            # tiny fused reductions over the PCA/proj matmul results, in the
            # gap while tile 7's halves arrive
            vector.wait_ge(s_pe, 1)
            nc.vector.tensor_copy(G_sb[:, :], psum_G[:, :])
            nc.vector.scalar_tensor_tensor(
                out=scr_m[:, :], in0=psum_M[:, :], scalar=1.0, in1=rsra_sb,
                op0=bypass, op1=mult, accum_out=S[:E, 9:10],
            )
            nc.vector.scalar_tensor_tensor(
                out=scr_i[:, :], in0=psum_L[:, :], scalar=1.0, in1=G_sb[:, :],
                op0=bypass, op1=mult, accum_out=S[:I, 10:11],
            )
            nc.vector.scalar_tensor_tensor(
                out=scr_i[:, :], in0=G_sb[:, :], scalar=1.0, in1=G_sb[:, :],
                op0=bypass, op1=mult, accum_out=S[:I, 11:12],
            ).then_inc(s_vfin, 1)
            # tile 7 halves: first half's square goes back to ACT (s_sub=8),
            # second half is fully handled here so the tail has no hop
            vector.wait_ge(s_x[7], 32)
            nc.vector.tensor_sub(
                xb[7][:, 0:DH], xb[7][:, 0:DH], db[7][:, 0:DH]
            ).then_inc(s_sub, 1)
            vector.wait_ge(s_x[8], 32)
            nc.vector.tensor_sub(xb[7][:, DH:D], xb[7][:, DH:D], db[7][:, DH:D])
            nc.vector.scalar_tensor_tensor(
                out=scr_e[:, 0:DH], in0=xb[7][:, DH:D], scalar=1.0,
                in1=xb[7][:, DH:D], op0=bypass, op1=mult,
                accum_out=S[:, 13:14],
            ).then_inc(s_vfin, 1)

        @block.tensor
        def _(tensor):
            tensor.wait_ge(s_small, 16)
            for t in range(RT):
                nc.tensor.matmul(
                    psum_M[:, :], lhsT=enc_t(t), rhs=lat_t(t),
                    start=(t == 0), stop=(t == RT - 1),
                )
            for t in range(RT):
                nc.tensor.matmul(
                    psum_L[:, :], lhsT=lat_t(t), rhs=lat_t(t),
                    start=(t == 0), stop=(t == RT - 1),
                )
            nc.tensor.matmul(
                psum_G[:, :], lhsT=rsra_sb, rhs=rsra_sb, start=True, stop=True
            ).then_inc(s_pe, 1)

    return nc


def kernel(x, encoded, latent, decoded, rsrA):
    global _NC, LAST_RESULT
    if _NC is None:
        _NC = _build_nc()

    x = np.ascontiguousarray(x, dtype=np.float32)
    decoded = np.ascontiguousarray(decoded, dtype=np.float32)
    encoded = np.ascontiguousarray(encoded, dtype=np.float32)
    latent = np.ascontiguousarray(latent, dtype=np.float32)
    rsrA = np.ascontiguousarray(rsrA, dtype=np.float32)

    in_maps = []
    for c in range(N_CORES):
        sl = slice(c * R, (c + 1) * R)
        pk = np.concatenate(
            [
                encoded[sl].reshape(P, ENC_W),
                latent[sl].reshape(P, LAT_W),
                rsrA,
            ],
            axis=1,
        )
        in_maps.append({"x": x[sl], "dec": decoded[sl], "pack": pk})

    res = run_bass_kernel_spmd(_NC, in_maps, core_ids=list(range(N_CORES)), trace=TRACE)
    LAST_RESULT = res

    o = np.stack([r["out"] for r in res.results]).astype(np.float64)  # [8,128,16]
    cols = o.sum(axis=(0, 1))  # [16]
    s_recon = cols[0:8].sum() + cols[13]
    s_enc2 = cols[8]
    s_cross = cols[9]
    s_zsq = cols[10]
    g2 = o[0, :, 11].sum()
    ra2 = o[0, :, 12].sum()

    pca_sq = s_enc2 - 2.0 * s_cross + s_zsq
    proj_sq = g2 - 2.0 * ra2 + float(I)
    loss = s_recon / B + 1.1 * pca_sq / B + 0.1 * proj_sq / (I * I)
    return np.asarray(loss, dtype=np.float32)
